# revision 2
# baseline (speedup 1.0000x reference)
"""CLIP attention (B=8, S=1024, H=1024, 16 heads) on 8 TRN2 NeuronCores.

Sharding: data-parallel over batch — core b computes attention for x[b].

v2 pipelined design (vs v1):
  - weights DMA straight into f32r tiles via bitcast (no DVE cast staging)
  - x -> xT PE transposes in f32r (1.5 cyc/row), pipelined behind x DMA
  - V path in bf16 (vp, pt, mergedT, wo) — halves SBUF + enables 1024 moving
  - attention inner loop at (qh, kk) half-tile granularity: sp [128, 1024]
    (even|odd head q-half) double-buffered -> exp never blocks the next
    scores matmuls (v1 stalled the PE ~1.1us per k-tile on a single sp)
  - up_e/up_o are [65, 512] (1 PSUM bank each, ones-column trick for the
    softmax denominator), freeing banks so the NEXT pair's Q/K projection
    matmuls interleave into the attention PE stream (2 per unit) instead
    of running as a serial ACT-idle prefix
  - per-pair normalization: broadcast r then reciprocal on [128, S]
    (v1 did reciprocal on [14, S]: 14 of 128 lanes, 6.5us each)
  - per-chunk DMA issue order: x first, then pair-0 Wq/Wk cols, then Wv
    rows, then remaining Wq/Wk, then Wo (needed last)
"""

import numpy as np

B = 8
S = 1024
H = 1024
NH = 16
D = 64
P = 128
NT = 8          # number of 128-tiles along S or H
SCALE = 0.125   # 1/sqrt(64)

_CACHE = {}


def _build():
    import concourse.bacc as bacc
    import concourse.mybir as mybir
    import concourse.tile as tile
    from concourse.masks import make_identity
    from contextlib import ExitStack

    F32 = mybir.dt.float32
    F32R = mybir.dt.float32r
    BF16 = mybir.dt.bfloat16
    EXP = mybir.ActivationFunctionType.Exp

    nc = bacc.Bacc(None)
    x = nc.dram_tensor("x", [S, H], F32, kind="ExternalInput")
    wq = nc.dram_tensor("Wq", [H, H], F32, kind="ExternalInput")
    wk = nc.dram_tensor("Wk", [H, H], F32, kind="ExternalInput")
    wv = nc.dram_tensor("Wv", [H, H], F32, kind="ExternalInput")
    wo = nc.dram_tensor("Wo", [H, H], F32, kind="ExternalInput")
    bq = nc.dram_tensor("bq", [H], F32, kind="ExternalInput")
    bk = nc.dram_tensor("bk", [H], F32, kind="ExternalInput")
    bv = nc.dram_tensor("bv", [H], F32, kind="ExternalInput")
    bo = nc.dram_tensor("bo", [H], F32, kind="ExternalInput")
    ident = nc.dram_tensor("ident", [P, P], F32, kind="ExternalInput")
    out = nc.dram_tensor("out", [S, H], F32, kind="ExternalOutput")
    rscr = nc.dram_tensor("rscr", [NH, S], F32)   # scratch for r broadcast

    with tile.TileContext(nc) as tc, ExitStack() as ctx:
        pers = ctx.enter_context(tc.tile_pool(name="pers", bufs=1))
        xT = pers.tile([P, NT, S], F32R, name="xT")
        wq_sb = pers.tile([P, NT, P], F32R, name="wq_sb", tag="wq0")
        wk_sb = pers.tile([P, NT, P], F32R, name="wk_sb", tag="wk0")
        vp = pers.tile([P, NT, NH * (D + 1)], BF16, name="vp")
        mergedT = pers.tile([P, NT, S], BF16, name="mergedT")
        wo_sb = pers.tile([P, NT, H], BF16, name="wo_sb")

        small = ctx.enter_context(tc.tile_pool(name="small", bufs=1))
        bq_sb = small.tile([P, NT], F32, name="bq_sb")
        bk_sb = small.tile([P, NT], F32, name="bk_sb")
        bv_bc = small.tile([P, H], F32, name="bv_bc")
        bo_bc = small.tile([P, H], F32, name="bo_bc")
        ones16 = small.tile([P, NH], F32, name="ones16")

        # wq/wk column-chunks for pairs 1..7 land in a ring; pair 0 goes to
        # the persistent wq_sb/wk_sb above so it can load first.
        wqk = ctx.enter_context(tc.tile_pool(name="wqk", bufs=2))
        qkpool = ctx.enter_context(tc.tile_pool(name="qk", bufs=2))
        ptpool = ctx.enter_context(tc.tile_pool(name="ptp", bufs=4))
        rbpool = ctx.enter_context(tc.tile_pool(name="rb", bufs=2))
        rppool = ctx.enter_context(tc.tile_pool(name="rp", bufs=1))
        wostage = ctx.enter_context(tc.tile_pool(name="wost", bufs=1))

        def load_wqk_cols(hp):
            """DMA the [H, 128] column slice of Wq/Wk for pair hp into
            [128(part=row%128), NT(row//128), 128] f32r tiles."""
            if hp == 0:
                q_t, k_t = wq_sb, wk_sb
            else:
                q_t = wqk.tile([P, NT, P], F32R, tag="wqc", name=f"wqc{hp}")
                k_t = wqk.tile([P, NT, P], F32R, tag="wkc", name=f"wkc{hp}")
            for src, dst in ((wq, q_t), (wk, k_t)):
                nc.sync.dma_start(
                    dst[:],
                    src[:, P * hp:P * (hp + 1)]
                    .rearrange("(kk p) c -> p kk c", p=P)
                    .bitcast(F32R))
            return q_t, k_t

        # ---- DMA issue order: smalls, x (in loop), pair-0 wq/wk, wv ----
        nc.sync.dma_start(bq_sb[:], bq.rearrange("(r p) -> p r", p=P))
        nc.sync.dma_start(bk_sb[:], bk.rearrange("(r p) -> p r", p=P))
        nc.sync.dma_start(bv_bc[:], bv[None, :].to_broadcast((P, H)))
        nc.sync.dma_start(bo_bc[:], bo[None, :].to_broadcast((P, H)))
        nc.vector.memset(ones16[:], 1.0)

        # ---- phase 0: x -> xT (pipelined), then V-proj per m-chunk ----
        with tc.tile_pool(name="wvpool", bufs=1) as wvpool, \
             tc.tile_pool(name="xstage", bufs=3) as xstage, \
             tc.tile_pool(name="idpool", bufs=1) as idpool, \
             tc.tile_pool(name="tpsum", bufs=4, space="PSUM") as tpsum, \
             tc.tile_pool(name="vpsum", bufs=2, space="PSUM") as vpsum:
            identity = idpool.tile([P, P], F32R, name="identity")
            nc.sync.dma_start(identity[:], ident[:, :].bitcast(F32R))
            identity_r = identity[:]

            # x chunks first (critical path), then pair-0 qk cols, then wv
            xs_tiles = []
            for m in range(NT):
                xs = xstage.tile([P, H], F32R, tag="xs", name=f"xs{m}")
                nc.sync.dma_start(xs[:], x[P * m:P * (m + 1), :].bitcast(F32R))
                xs_tiles.append(xs)
            wqk0 = load_wqk_cols(0)
            wv_sb = wvpool.tile([P, NT, H], F32R, name="wv_sb")
            for kk in range(NT):
                nc.sync.dma_start(
                    wv_sb[:, kk, :],
                    wv[P * kk:P * (kk + 1), :].bitcast(F32R))

            def v_proj(m):
                ps = vpsum.tile([P, S], F32, tag="ppv", name=f"ppv{m}")
                for kk in range(NT):
                    for n in range(2):
                        nc.tensor.matmul(
                            ps[:, 512 * n:512 * (n + 1)],
                            xT[:, kk, P * m:P * (m + 1)],
                            wv_sb[:, kk, 512 * n:512 * (n + 1)],
                            start=(kk == 0), stop=(kk == NT - 1))
                vview = vp[:, m, :].rearrange("p (h d) -> p h d", d=D + 1)
                nc.vector.tensor_add(
                    vview[:, :, 0:D],
                    ps[:].rearrange("p (h d) -> p h d", d=D),
                    bv_bc[:].rearrange("p (h d) -> p h d", d=D))
                nc.vector.tensor_copy(vview[:, :, D:D + 1],
                                      ones16[:].unsqueeze(2))

            for m in range(NT):
                xs = xs_tiles[m]
                for r in range(NT):
                    tp = tpsum.tile([P, P], F32R, tag="tp", name=f"tp{m}_{r}")
                    nc.tensor.transpose(tp[:], xs[:, P * r:P * (r + 1)],
                                        identity_r)
                    # alternate evac engine: ACT is idle in this phase
                    if r % 2 == 0:
                        nc.vector.tensor_copy(xT[:, r, P * m:P * (m + 1)],
                                              tp[:])
                    else:
                        nc.scalar.copy(xT[:, r, P * m:P * (m + 1)], tp[:])
                if m >= 1:
                    v_proj(m - 1)   # one chunk behind the transposes
            v_proj(NT - 1)

        # ---- attention: per head-pair, with next pair's Q/K interleaved ----
        with tc.tile_pool(name="qkpsum", bufs=2, space="PSUM") as qkpsum, \
             tc.tile_pool(name="spsum", bufs=2, space="PSUM") as spsum, \
             tc.tile_pool(name="upsum", bufs=1, space="PSUM") as upsum:

            def qk_proj_thunks(hp, w_cols):
                """Yield thunks: 32 matmuls + 4 bias evacs for pair hp."""
                qt_n = qkpool.tile([P, S], F32R, tag="qt", name=f"qt{hp}")
                kt_n = qkpool.tile([P, S], F32R, tag="kt", name=f"kt{hp}")
                thunks = []
                for w_t, dst, b_sb in ((w_cols[0], qt_n, bq_sb),
                                       (w_cols[1], kt_n, bk_sb)):
                    for n in range(2):
                        def group(w_t=w_t, dst=dst, b_sb=b_sb, n=n):
                            qps = qkpsum.tile([P, 512], F32, tag="qk",
                                              name=f"qk{hp}_{n}")
                            for kk in range(NT):
                                def mm(qps=qps, w_t=w_t, kk=kk, n=n):
                                    nc.tensor.matmul(
                                        qps[:],
                                        w_t[:, kk, :],
                                        xT[:, kk, 512 * n:512 * (n + 1)],
                                        start=(kk == 0), stop=(kk == NT - 1))
                                yield mm
                            def bias(qps=qps, dst=dst, b_sb=b_sb, n=n):
                                nc.vector.tensor_scalar_add(
                                    dst[:, 512 * n:512 * (n + 1)], qps[:],
                                    b_sb[:, hp:hp + 1])
                            yield bias
                        thunks.extend(group())
                return qt_n, kt_n, thunks

            # pair 0's projection runs serially before the loop
            qt_cur, kt_cur, th0 = qk_proj_thunks(0, wqk0)
            for t in th0:
                t()

            for hp in range(NT):
                he, ho = 2 * hp, 2 * hp + 1
                # stage next pair's weights + projection thunks
                pending = []
                if hp + 1 < NT:
                    w_cols = load_wqk_cols(hp + 1)
                    qt_nxt, kt_nxt, pending = qk_proj_thunks(hp + 1, w_cols)
                    pending = list(pending)
                # stage one wo row-chunk per pair (DVE cast f32->bf16)
                wos = wostage.tile([P, H], F32, tag="wos", name=f"wos{hp}")
                nc.sync.dma_start(wos[:], wo[P * hp:P * (hp + 1), :])

                r_e = rppool.tile([1, S], F32, tag="rpe", name=f"rpe{hp}")
                r_o = rppool.tile([1, S], F32, tag="rpo", name=f"rpo{hp}")
                up_e = upsum.tile([D + 1, 512], F32, tag="upe", name=f"upe{hp}")
                up_o = upsum.tile([D + 1, 512], F32, tag="upo", name=f"upo{hp}")

                def u_mms(pt, kk, qh):
                    nc.tensor.matmul(
                        up_e[:], vp[:, kk, (D + 1) * he:(D + 1) * (he + 1)],
                        pt[:, 0:512],
                        start=(kk == 0), stop=(kk == NT - 1))
                    nc.tensor.matmul(
                        up_o[:], vp[:, kk, (D + 1) * ho:(D + 1) * (ho + 1)],
                        pt[:, 512:1024],
                        start=(kk == 0), stop=(kk == NT - 1))

                for qh in range(2):
                    prev = None
                    for kk in range(NT):
                        sph = spsum.tile([P, 1024], F32, tag="sp",
                                         name=f"sp{hp}_{qh}_{kk}")
                        nc.tensor.matmul(
                            sph[:, 0:512],
                            kt_cur[0:D, P * kk:P * (kk + 1)],
                            qt_cur[0:D, 512 * qh:512 * (qh + 1)],
                            start=True, stop=True)
                        nc.tensor.matmul(
                            sph[:, 512:1024],
                            kt_cur[D:P, P * kk:P * (kk + 1)],
                            qt_cur[D:P, 512 * qh:512 * (qh + 1)],
                            start=True, stop=True)
                        pth = ptpool.tile([P, 1024], BF16, tag="pt",
                                          name=f"pt{hp}_{qh}_{kk}")
                        nc.scalar.activation(pth[:], sph[:], EXP, scale=SCALE)
                        if prev is not None:
                            u_mms(prev, kk - 1, qh)
                        # drip next pair's projection into the PE stream
                        for _ in range(3):
                            if pending:
                                pending.pop(0)()
                        prev = pth
                    u_mms(prev, NT - 1, qh)

                    # evacuate U^T (unnormalized) and the r rows
                    nc.vector.tensor_copy(
                        mergedT[0:D, hp, 512 * qh:512 * (qh + 1)], up_e[0:D, :])
                    nc.vector.tensor_copy(
                        mergedT[D:P, hp, 512 * qh:512 * (qh + 1)], up_o[0:D, :])
                    nc.vector.tensor_copy(
                        r_e[0:1, 512 * qh:512 * (qh + 1)], up_e[D:D + 1, :])
                    nc.vector.tensor_copy(
                        r_o[0:1, 512 * qh:512 * (qh + 1)], up_o[D:D + 1, :])

                while pending:
                    pending.pop(0)()

                # normalize this pair: bounce r via DRAM for the partition
                # broadcast, reciprocal + multiply on full 128 lanes
                nc.sync.dma_start(rscr[2 * hp:2 * hp + 1, :], r_e[:])
                nc.sync.dma_start(rscr[2 * hp + 1:2 * hp + 2, :], r_o[:])
                rb = rbpool.tile([P, S], F32, tag="rb", name=f"rb{hp}")
                nc.sync.dma_start(
                    rb[0:D, :], rscr[2 * hp, :][None, :].to_broadcast((D, S)))
                nc.sync.dma_start(
                    rb[D:P, :],
                    rscr[2 * hp + 1, :][None, :].to_broadcast((D, S)))
                nc.vector.reciprocal(rb[:], rb[:])
                nc.vector.tensor_mul(mergedT[:, hp, :], mergedT[:, hp, :],
                                     rb[:])
                # wo chunk cast (DVE) — spread across pairs
                nc.vector.tensor_copy(wo_sb[:, hp, :], wos[:])

                if hp + 1 < NT:
                    qt_cur, kt_cur = qt_nxt, kt_nxt

        # ---- output projection ----
        with tc.tile_pool(name="opsum", bufs=4, space="PSUM") as opsum, \
             tc.tile_pool(name="ostage", bufs=4) as ostage:
            for q in range(NT):
                for n in range(2):
                    ps = opsum.tile([P, 512], F32, tag="op", name=f"op{q}_{n}")
                    for r in range(NT):
                        nc.tensor.matmul(
                            ps[:],
                            mergedT[:, r, P * q:P * (q + 1)],
                            wo_sb[:, r, 512 * n:512 * (n + 1)],
                            start=(r == 0), stop=(r == NT - 1))
                    os_t = ostage.tile([P, 512], F32, tag="os",
                                       name=f"os{q}_{n}")
                    nc.vector.tensor_add(os_t[:], ps[:],
                                         bo_bc[:, 512 * n:512 * (n + 1)])
                    nc.sync.dma_start(
                        out[P * q:P * (q + 1), 512 * n:512 * (n + 1)], os_t[:])

    nc.finalize()
    return nc


def _in_maps(inputs):
    x = np.ascontiguousarray(np.asarray(inputs["x"], dtype=np.float32))
    eye = np.eye(P, dtype=np.float32)
    common = {k: np.ascontiguousarray(np.asarray(inputs[k], dtype=np.float32))
              for k in ("Wq", "Wk", "Wv", "Wo", "bq", "bk", "bv", "bo")}
    return [{"x": x[b], "ident": eye, **common} for b in range(B)]


def _gather(res, inputs):
    return np.stack([res.results[b]["out"] for b in range(B)]).astype(np.float32)


def kernel(**inputs):
    from concourse.bass_utils import run_bass_kernel_spmd

    nc = _CACHE.get("nc")
    if nc is None:
        nc = _CACHE["nc"] = _build()

    in_maps = _in_maps(inputs)
    res = run_bass_kernel_spmd(nc, in_maps, list(range(B)))
    return _gather(res, inputs)

